# revision 1
# baseline (speedup 1.0000x reference)
"""Bass/Trainium2 kernel for the BiLSTM tagger problem.

Self-contained: builds an SPMD bass program (same program on all 8 cores,
data-parallel over the batch: 16 sentences/core), runs it via
run_bass_kernel_spmd, and gathers the full [128, 256, 50] output.

Per-core plan (Bl=16 sentences, T=256):
  tokens are flattened t-major: F = t*16 + b  (4096 tokens, 32 tiles of 128)
  E  : gather embeddings (indirect DMA) + PE-transpose -> embT [128(E), 4096]
  L1 : 256 steps x 2 cells (fwd, bwd) interleaved.  Per cell-step:
         gates[16,1024] = embT_t.T@W1ihT + ones@b1 + h0T.T@W1hhT0 + h1T.T@W1hhT1
         (PSUM accumulation; gate order [g,i,f,o], g-rows pre-scaled x2)
         sg = sigmoid(gates)            (one ACT op; tanh(g) = 2*sig(2g)-1)
         tg = 2*sg_g - 1                (DVE tensor_scalar)
         a  = sg_i * tg ; b = sg_f * c ; c' = a + b
         tc = tanh(c') ; h = sg_o * tc
         hT = PE-transpose(h) -> h1T history [128, 2*4096] (chunks interleaved)
  P2 : pre2 = [h1f,h1b] @ W2ihT + b2 -> DRAM   (M=128 token tiles)
  L2 : like L1 but gates = inject(pre2[t]) + h@W2hhT
  OUT: tag logits = [h2f,h2b] @ woutT + bout -> out [4096, 50]
"""

import os
import numpy as np
import ml_dtypes

B, T_FULL = 128, 256
PHASES = os.environ.get("K_PHASES", "full")
BF16 = os.environ.get("K_BF16", "0") == "1"
F32R = os.environ.get("K_F32R", "1") == "1" and not BF16
SIGMERGE = os.environ.get("K_SIGMERGE", "0") == "1"
B_GPS = os.environ.get("K_BGPS", "0") == "1"
WBUFS = int(os.environ.get("K_WBUFS", "2"))
PREBUFS = int(os.environ.get("K_PREBUFS", "3"))
NSPLIT = int(os.environ.get("K_NSPLIT", "4"))
BF16_HOST = BF16
VOCAB, EMB, HID, TAGS = 50000, 128, 256, 50
NCORES = 8
BL = B // NCORES            # 16 sentences per core
G4 = 4 * HID                # 1024
F32 = None                  # set lazily (mybir.dt.float32)


def _patched_tile_context(nc):
    """TileContext whose final drain splits sem waits across nops (this
    walrus build allows only one sync wait on control instructions)."""
    import concourse.tile as tile
    from concourse import mybir

    class PatchedTileContext(tile.TileContext):
        MAX_W = 1       # control insts (nop/drain) + PE (ldweights encoding)
        MAX_W_SOFT = int(os.environ.get("K_MAXW", "1"))  # other engines

        def _add_instruction(self, inst):
            si = inst.sync_info
            lim = self.MAX_W
            if inst.engine in (mybir.EngineType.PE, mybir.EngineType.SP):
                lim = self.MAX_W
            elif not isinstance(inst, (mybir.InstNoOp, mybir.InstDrain)):
                lim = self.MAX_W_SOFT
            if si is not None and si.on_wait and len(si.on_wait) > lim:
                waits = list(si.on_wait)
                si.on_wait = waits[-lim:]
                rest = waits[:-lim]
                while rest:
                    nop = mybir.InstNoOp(
                        name=self.nc.get_next_instruction_name(),
                        ins=[], outs=[])
                    nop.engine = inst.engine
                    nop.sync_info = mybir.SyncInfo(
                        on_wait=rest[:self.MAX_W], on_update=[])
                    rest = rest[self.MAX_W:]
                    super()._add_instruction(nop)
            super()._add_instruction(inst)

        def _drain_and_barrier(self, tick_clock, wait_clock):
            nop_inst = self.nc.sync.nop()
            wait_clock.add_sem_waits(
                nop_inst.ins, tile.ScopedClock({None: tick_clock.global_clock})
            )
            si = nop_inst.ins.sync_info
            waits = list(si.on_wait) if si is not None else []
            MAX_W = 1
            if len(waits) > MAX_W:
                si.on_wait = waits[:MAX_W]
                rest = waits[MAX_W:]
                while rest:
                    extra = self.nc.sync.nop()
                    extra.ins.sync_info = mybir.SyncInfo(
                        on_wait=rest[:MAX_W], on_update=[]
                    )
                    rest = rest[MAX_W:]
            self.nc.sync.drain()
            self.nc.all_engine_barrier()
            assert self.sems is not None
            popped = self.nc._tile_sem_poison_stack.pop()
            assert popped is self._sem_poison
            self.nc.clear_and_free_semaphores(list(self.sems.allocated().values()))
            self.nc.all_engine_barrier()

    return PatchedTileContext(nc)


def build_program(T=T_FULL):
    import concourse.bass as bass
    import concourse.mybir as mybir

    f32 = mybir.dt.float32
    i32 = mybir.dt.int32
    f32r = mybir.dt.float32r
    # hdt: h-history + recurrent/projection weights; adt: other mm operands
    if F32R:
        hdt = f32r
        adt = f32r
    else:
        hdt = mybir.dt.bfloat16 if BF16 else f32
        adt = f32

    def rc(ap):
        return ap   # f32r handled via native tensor dtypes now
    SIG = mybir.ActivationFunctionType.Sigmoid
    TANH = mybir.ActivationFunctionType.Tanh
    MUL = mybir.AluOpType.mult
    ADD = mybir.AluOpType.add

    NTOK = BL * T
    NTT = NTOK // 128       # token tiles

    nc = bass.Bass()

    # ---------------- I/O ----------------
    sent = nc.dram_tensor("sent", [128, NTT], i32, kind="ExternalInput")
    emb_d = nc.dram_tensor("emb", [VOCAB, EMB], f32, kind="ExternalInput")
    ident16_d = nc.dram_tensor("ident16", [16, 16], f32, kind="ExternalInput")
    ident128_d = nc.dram_tensor("ident128", [128, 128], f32, kind="ExternalInput")
    ones_d = nc.dram_tensor("ones_row", [1, 128], adt, kind="ExternalInput")
    ident16h_d = nc.dram_tensor("ident16h", [16, 16], mybir.dt.bfloat16,
                                kind="ExternalInput")
    ident16r_d = nc.dram_tensor("ident16r", [16, 16], adt,
                                kind="ExternalInput")
    w_in = {}
    for cell, din in (("1f", EMB), ("1b", EMB), ("2f", 2 * HID), ("2b", 2 * HID)):
        wdt = adt if din == EMB else hdt
        w_in[f"wih{cell}"] = nc.dram_tensor(f"wih{cell}", [din, G4], wdt,
                                            kind="ExternalInput")
        w_in[f"whh{cell}"] = nc.dram_tensor(f"whh{cell}", [HID, G4], hdt,
                                            kind="ExternalInput")
        w_in[f"b{cell}"] = nc.dram_tensor(f"b{cell}", [1, G4], adt,
                                          kind="ExternalInput")
    wout_d = nc.dram_tensor("woutT", [2 * HID, TAGS], hdt, kind="ExternalInput")
    bout_d = nc.dram_tensor("bout", [1, TAGS], adt, kind="ExternalInput")
    out_d = nc.dram_tensor("out", [NTOK, TAGS], f32, kind="ExternalOutput")

    tc = _patched_tile_context(nc)
    with tc:
        import concourse.tile as tile  # noqa

        with tc.tile_pool(name="const", bufs=1) as cp, \
                tc.tile_pool(name="dram", bufs=1, space="DRAM") as dramp:
            ident16 = cp.tile([16, 16], f32)
            nc.sync.dma_start(ident16[:], ident16_d[:])
            ident128 = cp.tile([128, 128], f32)
            nc.sync.dma_start(ident128[:], ident128_d[:])
            ones_row = cp.tile([1, 128], adt)
            nc.sync.dma_start(ones_row[:], ones_d[:])
            ident16h = cp.tile([16, 16], mybir.dt.bfloat16)
            nc.sync.dma_start(ident16h[:], ident16h_d[:])
            ident16r = cp.tile([16, 16], adt)
            nc.sync.dma_start(ident16r[:], ident16r_d[:])
            # 2*HID=512 partitions won't fit one tile; load as 4 chunks
            wout_ch = []
            for k in range(4):
                wt = cp.tile([128, TAGS], hdt, tag=f"wout{k}", name=f"swout{k}")
                nc.sync.dma_start(wt[:], wout_d[128 * k:128 * (k + 1), :])
                wout_ch.append(wt)
            bout = cp.tile([1, TAGS], adt)
            nc.sync.dma_start(bout[:], bout_d[:])
            biases = {}
            for cell in ("1f", "1b", "2f", "2b"):
                bt = cp.tile([1, G4], adt, tag=f"b{cell}", name=f"sb{cell}")
                nc.sync.dma_start(bt[:], w_in[f"b{cell}"][:])
                biases[cell] = bt

            pre2_d = {
                "2f": dramp.tile([NTOK, G4], adt, name="pre2f_d"),
                "2b": dramp.tile([NTOK, G4], adt, name="pre2b_d"),
            }

            PSUM_PRE = False  # DMA cannot write PSUM in this stack

            def lstm_layer(tc, layer, pre_dram, whh, hT_hist, ident16, identh, identr, T):
                """T steps x 2 cells (f fwd, b bwd) with batch-16 chains.
                gates = pre[t] (DMA'd into PSUM, or identity-injected) +
                        hT0.T@WhhT0 + hT1.T@WhhT1   (PSUM accum)
                pointwise: sigmoid trick for tanh(g); tail transposes c and
                sigma_o, then h is produced directly in hT layout."""
                cells = ("f", "b")
                with tc.tile_pool(name=f"l{layer}_work", bufs=WBUFS) as lp, \
                        tc.tile_pool(name=f"l{layer}_psum", bufs=2,
                                     space="PSUM") as pp:
                    c_prev = {cc: None for cc in cells}
                    for s in range(T):
                        for cc in cells:
                            t = s if cc == "f" else T - 1 - s
                            cell = f"{layer}{cc}"
                            gp = pp.tile([16, G4], f32, tag=f"g{cc}", bufs=1)
                            rows = slice(16 * t, 16 * (t + 1))
                            if s == 0:
                                # gates = pre only; start=True primes has_written
                                pt = lp.tile([16, G4], adt, tag=f"pre{cc}",
                                             bufs=PREBUFS, name=f"pre{cc}")
                                nc.sync.dma_start(pt[:], pre_dram[cell][rows, :])
                                for n in range(2):
                                    nsl = slice(512 * n, 512 * (n + 1))
                                    nc.tensor.matmul(
                                        gp[:, nsl], identr[:], pt[:, nsl],
                                        start=True, stop=True)
                            else:
                                tp_ = t + 1 if cc == "b" else t - 1
                                hb = 256 * (tp_ // 8) + 16 * (tp_ % 8)
                                if PSUM_PRE:
                                    nc.sync.dma_start(
                                        gp[:], pre_dram[cell][rows, :])
                                else:
                                    pt = lp.tile([16, G4], adt, tag=f"pre{cc}",
                                                 bufs=PREBUFS, name=f"pre{cc}")
                                    nc.sync.dma_start(
                                        pt[:], pre_dram[cell][rows, :])
                                for n in range(2):
                                    nsl = slice(512 * n, 512 * (n + 1))
                                    if not PSUM_PRE:
                                        nc.tensor.matmul(
                                            gp[:, nsl], identr[:], pt[:, nsl],
                                            start=True, stop=False)
                                    nc.tensor.matmul(
                                        gp[:, nsl],
                                        rc(hT_hist[cc][:, hb:hb + 16]),
                                        rc(whh[cell][0][:, nsl]),
                                        start=False, stop=False,
                                        skip_group_check=PSUM_PRE)
                                    nc.tensor.matmul(
                                        gp[:, nsl],
                                        rc(hT_hist[cc][:, hb + 128:hb + 144]),
                                        rc(whh[cell][1][:, nsl]),
                                        start=False, stop=True,
                                        skip_group_check=PSUM_PRE)
                            # ---- pointwise ----
                            pdt = mybir.dt.bfloat16 if BF16 else f32
                            SPLIT3 = os.environ.get("K_SPLIT3", "1") == "1"
                            if SPLIT3:
                                # sigma over [g,i,f] first (feeds the whole DVE
                                # chain); sigma(o) runs parallel with it
                                sgi = lp.tile([16, 768], pdt, tag=f"sgi{cc}",
                                              name=f"sgi{cc}")
                                nc.scalar.activation(sgi[:], gp[:, 0:768], SIG)
                                sfo = lp.tile([16, 512], pdt, tag=f"sfo{cc}",
                                              name=f"sfo{cc}")
                                nc.scalar.activation(
                                    sfo[:, 256:512], gp[:, 768:1024], SIG)
                            elif SIGMERGE:
                                sgall = lp.tile([16, G4], pdt, tag=f"sg{cc}",
                                                name=f"sg{cc}")
                                nc.scalar.activation(sgall[:], gp[:], SIG)
                                sgi = sgall[:, 0:512]
                                sfo = sgall[:, 512:1024]
                            else:
                                sgi = lp.tile([16, 512], pdt, tag=f"sgi{cc}")
                                nc.scalar.activation(sgi[:], gp[:, 0:512], SIG)
                                sfo = lp.tile([16, 512], pdt, tag=f"sfo{cc}")
                                nc.scalar.activation(sfo[:], gp[:, 512:1024], SIG)
                            tg = lp.tile([16, HID], pdt, tag=f"tg{cc}")
                            nc.vector.tensor_scalar(
                                tg[:], sgi[:, 0:HID], 2.0, -1.0, MUL, ADD)
                            a_t = lp.tile([16, HID], f32, tag=f"a{cc}")
                            nc.vector.tensor_tensor(
                                a_t[:], sgi[:, HID:2 * HID], tg[:], MUL)
                            if s == 0:
                                c_new = a_t
                            else:
                                b_t = lp.tile([16, HID], f32, tag=f"bb{cc}")
                                beng = nc.gpsimd if B_GPS else nc.vector
                                bsrc = (sgi[:, 512:768] if SPLIT3
                                        else sfo[:, 0:HID])
                                beng.tensor_tensor(
                                    b_t[:], bsrc, c_prev[cc][:], MUL)
                                c_new = lp.tile([16, HID], f32, tag=f"c{cc}",
                                                name=f"c{cc}")
                                nc.vector.tensor_tensor(
                                    c_new[:], a_t[:], b_t[:], ADD)
                            c_prev[cc] = c_new
                            # ---- transposed tail: hT = sigmoid(o).T * tanh(c).T
                            sop = pp.tile([128, 32], pdt, tag=f"so{cc}", bufs=1)
                            idt = identh if BF16 else ident16
                            nc.tensor.transpose(
                                sop[:, 0:16], sfo[:, 256:384], idt[:])
                            nc.tensor.transpose(
                                sop[:, 16:32], sfo[:, 384:512], idt[:])
                            soT = lp.tile([128, 32], pdt, tag=f"soT{cc}")
                            nc.vector.tensor_copy(soT[:], sop[:])
                            ctp = pp.tile([128, 32], f32, tag=f"ct{cc}", bufs=1)
                            nc.tensor.transpose(
                                ctp[:, 0:16], c_new[:, 0:128], ident16[:])
                            nc.tensor.transpose(
                                ctp[:, 16:32], c_new[:, 128:256], ident16[:])
                            tcT = lp.tile([128, 32], pdt, tag=f"tcT{cc}")
                            nc.scalar.activation(tcT[:], ctp[:], TANH)
                            base = 256 * (t // 8) + 16 * (t % 8)
                            nc.vector.tensor_tensor(
                                hT_hist[cc][:, base:base + 16],
                                soT[:, 0:16], tcT[:, 0:16], MUL)
                            nc.vector.tensor_tensor(
                                hT_hist[cc][:, base + 128:base + 144],
                                soT[:, 16:32], tcT[:, 16:32], MUL)

            def proj_gemm(tc, name, lhs_fn, nchunks, wih, bias_t, dst, NTT,
                          ones_row, order=None):
                """dst[128g:128g+128, :] = sum_k lhs_k.T @ wih[k] + ones x bias"""
                with tc.tile_pool(name=f"{name}w", bufs=3) as pw, \
                        tc.tile_pool(name=f"{name}p", bufs=2,
                                     space="PSUM") as pps:
                    for g in (order if order is not None else range(NTT)):
                        csl = slice(128 * g, 128 * (g + 1))
                        ps = pps.tile([128, G4], f32, tag="ps", name="ps")
                        for n in range(2):
                            nsl = slice(512 * n, 512 * (n + 1))
                            for k in range(nchunks):
                                nc.tensor.matmul(
                                    ps[:, nsl], rc(lhs_fn(g, k)),
                                    rc(wih[k][:, nsl]),
                                    start=(k == 0), stop=False)
                            nc.tensor.matmul(
                                ps[:, nsl], rc(ones_row[:1, :]),
                                rc(bias_t[:1, nsl]),
                                start=False, stop=True)
                        sb = pw.tile([128, G4], adt, tag="sb", name="sb")
                        nc.vector.tensor_copy(sb[:], ps[:])
                        nc.sync.dma_start(dst[csl, :], sb[:])

            # ================= E + P1 + L1 =================
            pre1_d = {
                "1f": dramp.tile([NTOK, G4], adt, name="pre1f_d"),
                "1b": dramp.tile([NTOK, G4], adt, name="pre1b_d"),
            }
            h1T = {}
            with tc.tile_pool(name="h1T", bufs=1) as p_h1:
                for cc in ("f", "b"):
                    h1T[cc] = p_h1.tile([128, 32 * T], hdt, tag=f"h1T{cc}",
                                        name=f"h1T{cc}")

                with tc.tile_pool(name="l1_fix", bufs=1) as p_l1:
                    # --- embedding gather + transpose ---
                    sidx = p_l1.tile([128, NTT], i32)
                    nc.sync.dma_start(sidx[:], sent[:, 0:NTT])
                    embT = p_l1.tile([128, NTOK], adt)
                    with tc.tile_pool(name="embp", bufs=3) as ep, \
                            tc.tile_pool(name="embpp", bufs=2,
                                         space="PSUM") as epp:
                        for g in range(NTT):
                            et = ep.tile([128, EMB], f32, tag="et")
                            nc.gpsimd.indirect_dma_start(
                                out=et[:], out_offset=None,
                                in_=emb_d[:],
                                in_offset=bass.IndirectOffsetOnAxis(
                                    ap=sidx[:, g:g + 1], axis=0),
                            )
                            etp = epp.tile([128, EMB], f32, tag="etp")
                            nc.tensor.transpose(etp[:], et[:], ident128[:])
                            nc.vector.tensor_copy(
                                embT[:, 128 * g:128 * (g + 1)], etp[:])

                    # --- L1 weights ---
                    w1ih, w1hh = {}, {}
                    for cell in ("1f", "1b"):
                        wt = p_l1.tile([EMB, G4], adt, tag=f"wih{cell}",
                                       name=f"swih{cell}")
                        nc.sync.dma_start(wt[:], w_in[f"wih{cell}"][:])
                        w1ih[cell] = wt
                        hh = []
                        for k in range(2):
                            ht = p_l1.tile([128, G4], hdt, tag=f"whh{cell}{k}",
                                           name=f"swhh{cell}{k}")
                            nc.sync.dma_start(
                                ht[:],
                                w_in[f"whh{cell}"][128 * k:128 * (k + 1), :])
                            hh.append(ht)
                        w1hh[cell] = hh

                    # --- P1: pre1 = embT.T @ W1ihT + b -> DRAM ---
                    for cell in ("1f", "1b"):
                        proj_gemm(
                            tc, f"p1{cell}",
                            lambda g, k, _c=cell: embT[:, 128 * g:128 * (g + 1)],
                            1, [w1ih[cell]], biases[cell], pre1_d[cell],
                            NTT, ones_row)

                    lstm_layer(tc, 1, pre1_d, w1hh, h1T, ident16, ident16h, ident16r, T)
                # p_l1 closed: embT + W1 freed

                # ================= P2 =================
                if PHASES == "el1":
                    return nc
                with tc.tile_pool(name="p2_fix", bufs=1) as p_p2:
                    w2ih = {}
                    for cell in ("2f", "2b"):
                        ch = []
                        for k in range(4):
                            wt = p_p2.tile([128, G4], hdt, tag=f"wih{cell}{k}",
                                           name=f"swih{cell}{k}")
                            nc.sync.dma_start(
                                wt[:],
                                w_in[f"wih{cell}"][128 * k:128 * (k + 1), :])
                            ch.append(wt)
                        w2ih[cell] = ch
                    # order by L2 consumption time: L2f needs tile g at
                    # step 8g, L2b needs it at step T-8-8g; edge tiles first
                    # so L2 starts as soon as L1 drains, middle tiles overlap
                    gorder = sorted(range(NTT),
                                    key=lambda g: min(8 * g, T - 8 - 8 * g))

                    def p2_lhs(g, k):
                        cc = "f" if k < 2 else "b"
                        cb = 256 * g + 128 * (k % 2)
                        return h1T[cc][:, cb:cb + 128]

                    for cell in ("2f", "2b"):
                        proj_gemm(tc, f"p2{cell}", p2_lhs, 4, w2ih[cell],
                                  biases[cell], pre2_d[cell], NTT, ones_row,
                                  order=gorder)
            # h1T freed here

            # ================= L2 + OUT =================
            if PHASES in ("el1", "el1p2"):
                return nc
            h2T = {}
            with tc.tile_pool(name="l2_fix", bufs=1) as p_l2:
                for cc in ("f", "b"):
                    h2T[cc] = p_l2.tile([128, 32 * T], hdt, tag=f"h2T{cc}", name=f"h2T{cc}")
                w2hh = {}
                for cell in ("2f", "2b"):
                    hh = []
                    for k in range(2):
                        ht = p_l2.tile([128, G4], hdt, tag=f"whh{cell}{k}", name=f"swhh{cell}{k}")
                        nc.sync.dma_start(
                            ht[:], w_in[f"whh{cell}"][128 * k:128 * (k + 1), :])
                        hh.append(ht)
                    w2hh[cell] = hh

                lstm_layer(tc, 2, pre2_d, w2hh, h2T, ident16, ident16h, ident16r, T)

                # --------- output projection ---------
                with tc.tile_pool(name="outw", bufs=3) as ow, \
                        tc.tile_pool(name="outp", bufs=2, space="PSUM") as op:
                    for g in range(NTT):
                        csl = slice(128 * g, 128 * (g + 1))
                        lhs = []
                        for cc in ("f", "b"):
                            for k in range(2):
                                cb = 256 * g + 128 * k
                                lhs.append(h2T[cc][:, cb:cb + 128])
                        ps = op.tile([128, TAGS], f32, tag="ops")
                        for k in range(4):
                            nc.tensor.matmul(ps[:], lhs[k], wout_ch[k][:],
                                             start=(k == 0), stop=False)
                        nc.tensor.matmul(ps[:], ones_row[:1, :], bout[:1, :],
                                         start=False, stop=True)
                        sb = ow.tile([128, TAGS], f32, tag="osb")
                        nc.vector.tensor_copy(sb[:], ps[:])
                        nc.sync.dma_start(out_d[csl, :], sb[:])

    return nc


def _prep_cell_weights(wih, whh, bih, bhh):
    """Permute gate rows i,f,g,o -> g,i,f,o ; scale g rows (and bias) by 2
    for the tanh(x)=2*sigmoid(2x)-1 trick; return (wihT, whhT, brow) f32."""
    H = HID
    idx = np.concatenate([np.arange(2 * H, 3 * H),      # g
                          np.arange(0, H),              # i
                          np.arange(H, 2 * H),          # f
                          np.arange(3 * H, 4 * H)])     # o
    scale = np.ones((4 * H, 1), np.float32)
    scale[0:H] = 2.0
    wih_p = wih[idx] * scale
    whh_p = whh[idx] * scale
    b_p = (bih + bhh)[idx] * scale[:, 0]
    return (np.ascontiguousarray(wih_p.T, np.float32),
            np.ascontiguousarray(whh_p.T, np.float32),
            np.ascontiguousarray(b_p[None, :], np.float32))


class Runner:
    """Build the SPMD program once; execute repeatedly on device-resident
    inputs (for clean timing, no donation so buffers are reusable)."""

    def __init__(self, nc, n_cores=NCORES):
        import jax
        import numpy as _np
        from jax.sharding import Mesh, PartitionSpec
        from jax.experimental.shard_map import shard_map
        import concourse.mybir as mybir
        from concourse import bass2jax as b2j

        b2j.install_neuronx_cc_hook()
        self.jax = jax
        self.nc = nc
        self.n_cores = n_cores
        partition_name = (nc.partition_id_tensor.name
                          if nc.partition_id_tensor else None)
        in_names, out_names, out_avals, zero_outs = [], [], [], []
        for alloc in nc.m.functions[0].allocations:
            if not isinstance(alloc, mybir.MemoryLocationSet):
                continue
            name = alloc.memorylocations[0].name
            if alloc.kind == "ExternalInput":
                if name != partition_name:
                    in_names.append(name)
            elif alloc.kind == "ExternalOutput":
                out_names.append(name)
                shape = tuple(alloc.tensor_shape)
                dtype = mybir.dt.np(alloc.dtype)
                out_avals.append(jax.core.ShapedArray(shape, dtype))
                zero_outs.append(_np.zeros(shape, dtype))
        self.n_params = len(in_names)
        self.in_names = list(in_names)
        self.out_names = list(out_names)
        self.out_avals = out_avals
        self.zero_outs = zero_outs
        all_in = in_names + out_names
        if partition_name is not None:
            all_in.append(partition_name)

        def _body(*args):
            operands = list(args)
            if partition_name is not None:
                operands.append(b2j.partition_id_tensor())
            outs = b2j._bass_exec_p.bind(
                *operands,
                out_avals=tuple(out_avals),
                in_names=tuple(all_in),
                out_names=tuple(out_names),
                lowering_input_output_aliases=(),
                sim_require_finite=True,
                sim_require_nnan=True,
                nc=nc,
            )
            return tuple(outs)

        devices = jax.devices()[:n_cores]
        self.mesh = Mesh(_np.asarray(devices), ("core",))
        in_specs = (PartitionSpec("core"),) * (self.n_params + len(out_names))
        out_specs = (PartitionSpec("core"),) * len(out_names)
        self.sharded = jax.jit(shard_map(_body, mesh=self.mesh,
                                         in_specs=in_specs,
                                         out_specs=out_specs, check_rep=False),
                               keep_unused=True)
        self.dev_args = None

    def put(self, in_maps):
        """Upload per-core input maps as device-sharded global arrays."""
        import numpy as _np
        from jax.sharding import NamedSharding, PartitionSpec
        jax = self.jax
        sh = NamedSharding(self.mesh, PartitionSpec("core"))
        args = []
        for name in self.in_names:
            g = _np.concatenate([_np.asarray(m[name]) for m in in_maps], axis=0)
            args.append(jax.device_put(g, sh))
        for z in self.zero_outs:
            g = _np.zeros((self.n_cores * z.shape[0],) + z.shape[1:], z.dtype)
            args.append(jax.device_put(g, sh))
        self.dev_args = args

    def run(self):
        outs = self.sharded(*self.dev_args)
        self.jax.block_until_ready(outs)
        return outs

    def results(self, outs):
        import numpy as _np
        res = []
        for c in range(self.n_cores):
            res.append({name: _np.asarray(outs[i]).reshape(
                (self.n_cores,) + self.out_avals[i].shape)[c]
                for i, name in enumerate(self.out_names)})
        return res

    def time_exec(self, iters=10):
        import time as _time
        self.run()  # warm
        best = float("inf")
        for _ in range(iters):
            t0 = _time.perf_counter()
            self.run()
            best = min(best, _time.perf_counter() - t0)
        return best


_RUNNERS = {}


def get_runner(T=T_FULL):
    if T not in _RUNNERS:
        _RUNNERS[T] = Runner(build_program(T))
    return _RUNNERS[T]


def make_in_maps(sentence, emb,
                 wih1f, whh1f, bih1f, bhh1f,
                 wih1b, whh1b, bih1b, bhh1b,
                 wih2f, whh2f, bih2f, bhh2f,
                 wih2b, whh2b, bih2b, bhh2b,
                 w_out, b_out, T=T_FULL):
    NTOK = BL * T
    NTT = NTOK // 128
    common = {
        "emb": np.asarray(emb, np.float32),
        "ident16": np.eye(16, dtype=np.float32),
        "ident16h": np.eye(16).astype(ml_dtypes.bfloat16),
        "ident16r": np.eye(16, dtype=np.float32),
        "ident128": np.eye(128, dtype=np.float32),
        "ones_row": np.ones((1, 128), np.float32),
        "woutT": (np.ascontiguousarray(np.asarray(w_out, np.float32).T)
                  .astype(ml_dtypes.bfloat16 if BF16_HOST else np.float32)),
        "bout": np.asarray(b_out, np.float32).reshape(1, TAGS),
    }
    for cell, (wi, wh, bi, bh) in {
        "1f": (wih1f, whh1f, bih1f, bhh1f),
        "1b": (wih1b, whh1b, bih1b, bhh1b),
        "2f": (wih2f, whh2f, bih2f, bhh2f),
        "2b": (wih2b, whh2b, bih2b, bhh2b),
    }.items():
        wihT, whhT, brow = _prep_cell_weights(
            np.asarray(wi, np.float32), np.asarray(wh, np.float32),
            np.asarray(bi, np.float32), np.asarray(bh, np.float32))
        if BF16_HOST:
            whhT = whhT.astype(ml_dtypes.bfloat16)
            if cell in ("2f", "2b"):
                wihT = wihT.astype(ml_dtypes.bfloat16)
        common[f"wih{cell}"] = wihT
        common[f"whh{cell}"] = whhT
        common[f"b{cell}"] = brow
    sentence = np.asarray(sentence)
    in_maps = []
    for c in range(NCORES):
        sl = sentence[c * BL:(c + 1) * BL, :T]
        flat = np.ascontiguousarray(sl.T).reshape(NTOK)
        sent_in = np.ascontiguousarray(
            flat.reshape(NTT, 128).T.astype(np.int32))
        m = dict(common)
        m["sent"] = sent_in
        in_maps.append(m)
    return in_maps


def kernel(sentence, emb,
           wih1f, whh1f, bih1f, bhh1f,
           wih1b, whh1b, bih1b, bhh1b,
           wih2f, whh2f, bih2f, bhh2f,
           wih2b, whh2b, bih2b, bhh2b,
           w_out, b_out, _T=T_FULL, _trace=False):
    T = _T
    rn = get_runner(T)
    in_maps = make_in_maps(sentence, emb,
                           wih1f, whh1f, bih1f, bhh1f,
                           wih1b, whh1b, bih1b, bhh1b,
                           wih2f, whh2f, bih2f, bhh2f,
                           wih2b, whh2b, bih2b, bhh2b,
                           w_out, b_out, T=T)
    rn.put(in_maps)
    outs = rn.run()
    res = rn.results(outs)
    NTOK = BL * T
    full = np.concatenate(
        [res[c]["out"].reshape(T, BL, TAGS).transpose(1, 0, 2)
         for c in range(NCORES)], axis=0)
    return full



# revision 20
# speedup vs baseline: 1.8243x; 1.8243x over previous
"""Bass/Trainium2 kernel for the BiLSTM tagger problem (transposed design).

Self-contained: builds an SPMD bass program (same program on all 8 cores,
data-parallel over the batch: 16 sentences/core), runs it via a bass2jax
shard_map runner, and gathers the full [128, 256, 50] output.

Layout: everything transposed — partition dim = feature dim, free dim =
(cell, h-chunk, batch).  Per core (Bl=16, T=256):

  E   : gather bf16 embeddings (indirect DMA, ends-inward tile order so L1
        can start immediately) + PE-transpose -> embT [128(E), 4096] bf16
  L1/L2 recurrence, fwd+bwd cells packed per step s (tf=s, tb=T-1-s):
        gatesT psum [128, 256] f32, col = gate(o,i,f,g)*64 + cc*32 + hc*16 + b
        bias  : 1 matmul  K=16 selector (WbT [16,128] @ S [16,256])
        pre   : L1: 16 mm N=16 (embT cols), L2: 64 mm N=16 (hist1 cols)
        hh    : 32 mm N=16 (lhsT = WhhT block [128,128], rhs = hT scratch)
        sgi   = sigmoid(gatesT[i,f,g])          (ACT; tanh via 2sig(2x)-1)
        sgo   = sigmoid(gatesT[o])              (ACT, off critical path)
        b     = sgi_f * c_prev                  (DVE tt)
        a'    = (sgi_g - 0.5) * sgi_i           (DVE stt)   [a = 2a']
        c     = 2*a' + b                        (DVE stt)
        tc'   = sigmoid(2c)                     (ACT, scale=2) [tanh(c)=2tc'-1]
        hT    = (tc' - 0.5) * sgo               (DVE stt -> scratch, feeds hh)
        hist_f/hist_b <- hT halves              (Pool copies, off the chain)
        hist stores h' = h/2; consumers (Whh, W2ih, wout) host-scaled x2.
  OUT : interleaved into the L2 loop as token tiles complete:
        per tile g: 8x(1+4) mm N=50 + Pool copies -> out [4096, 50] f32
"""

import os
import numpy as np
import ml_dtypes

B, T_FULL = 128, 256
VOCAB, EMB, HID, TAGS = 50000, 128, 256, 50
NCORES = 8
BL = B // NCORES            # 16 sentences per core
G4 = 4 * HID                # 1024
PSUM_BUFS = int(os.environ.get("K_PSUM_BUFS", "4"))
SG_BUFS = int(os.environ.get("K_SG_BUFS", "4"))


def _patched_tile_context(nc):
    """TileContext whose final drain splits sem waits across nops (this
    walrus build allows only one sync wait on control instructions)."""
    import concourse.tile as tile
    from concourse import mybir

    class PatchedTileContext(tile.TileContext):
        MAX_W = 1       # control insts (nop/drain) + PE (ldweights encoding)
        MAX_W_SOFT = int(os.environ.get("K_MAXW", "1"))  # other engines

        def _add_instruction(self, inst):
            si = inst.sync_info
            lim = self.MAX_W
            if inst.engine in (mybir.EngineType.PE, mybir.EngineType.SP):
                lim = self.MAX_W
            elif not isinstance(inst, (mybir.InstNoOp, mybir.InstDrain)):
                lim = self.MAX_W_SOFT
            if si is not None and si.on_wait and len(si.on_wait) > lim:
                waits = list(si.on_wait)
                si.on_wait = waits[-lim:]
                rest = waits[:-lim]
                while rest:
                    nop = mybir.InstNoOp(
                        name=self.nc.get_next_instruction_name(),
                        ins=[], outs=[])
                    nop.engine = inst.engine
                    nop.sync_info = mybir.SyncInfo(
                        on_wait=rest[:self.MAX_W], on_update=[])
                    rest = rest[self.MAX_W:]
                    super()._add_instruction(nop)
            super()._add_instruction(inst)

        def _drain_and_barrier(self, tick_clock, wait_clock):
            nop_inst = self.nc.sync.nop()
            wait_clock.add_sem_waits(
                nop_inst.ins, tile.ScopedClock({None: tick_clock.global_clock})
            )
            si = nop_inst.ins.sync_info
            waits = list(si.on_wait) if si is not None else []
            MAX_W = 1
            if len(waits) > MAX_W:
                si.on_wait = waits[:MAX_W]
                rest = waits[MAX_W:]
                while rest:
                    extra = self.nc.sync.nop()
                    extra.ins.sync_info = mybir.SyncInfo(
                        on_wait=rest[:MAX_W], on_update=[]
                    )
                    rest = rest[MAX_W:]
            self.nc.sync.drain()
            self.nc.all_engine_barrier()
            assert self.sems is not None
            popped = self.nc._tile_sem_poison_stack.pop()
            assert popped is self._sem_poison
            self.nc.clear_and_free_semaphores(list(self.sems.allocated().values()))
            self.nc.all_engine_barrier()

    return PatchedTileContext(nc)


def build_program(T=T_FULL):
    import concourse.bass as bass
    import concourse.mybir as mybir

    f32 = mybir.dt.float32
    i32 = mybir.dt.int32
    bf16 = mybir.dt.bfloat16
    SIG = mybir.ActivationFunctionType.Sigmoid
    MUL = mybir.AluOpType.mult
    ADD = mybir.AluOpType.add
    SUB = mybir.AluOpType.subtract

    NTOK = BL * T
    NTT = NTOK // 128       # token tiles (32)

    nc = bass.Bass()

    # ---------------- I/O ----------------
    sent = nc.dram_tensor("sent", [128, NTT], i32, kind="ExternalInput")
    emb_d = nc.dram_tensor("emb", [VOCAB, EMB], bf16, kind="ExternalInput")
    ident_d = nc.dram_tensor("ident128b", [128, 128], bf16, kind="ExternalInput")
    onescol_d = nc.dram_tensor("onescol", [1, 128], bf16, kind="ExternalInput")
    bsel_d = nc.dram_tensor("bsel", [16, 256], bf16, kind="ExternalInput")
    w_in = {}
    for cell in ("1f", "1b"):
        w_in[f"wih{cell}"] = nc.dram_tensor(f"wih{cell}", [EMB, G4], bf16,
                                            kind="ExternalInput")
    for cell in ("2f", "2b"):
        w_in[f"wih{cell}"] = nc.dram_tensor(f"wih{cell}", [2 * HID, G4], bf16,
                                            kind="ExternalInput")
    for cell in ("1f", "1b", "2f", "2b"):
        w_in[f"whh{cell}"] = nc.dram_tensor(f"whh{cell}", [HID, G4], bf16,
                                            kind="ExternalInput")
    wb_d = {1: nc.dram_tensor("wb1", [16, 128], bf16, kind="ExternalInput"),
            2: nc.dram_tensor("wb2", [16, 128], bf16, kind="ExternalInput")}
    wout_d = nc.dram_tensor("woutT", [2 * HID, TAGS], bf16, kind="ExternalInput")
    bout_d = nc.dram_tensor("bout", [1, TAGS], bf16, kind="ExternalInput")
    out_d = nc.dram_tensor("out", [TAGS, NTOK], f32, kind="ExternalOutput")

    tc = _patched_tile_context(nc)
    with tc:
        with tc.tile_pool(name="const", bufs=1) as cp:
            ident = cp.tile([128, 128], bf16)
            nc.sync.dma_start(ident[:], ident_d[:])
            onescol = cp.tile([1, 128], bf16)
            nc.sync.dma_start(onescol[:], onescol_d[:])
            bsel = cp.tile([16, 256], bf16)
            nc.sync.dma_start(bsel[:], bsel_d[:])
            NTT_ = BL * T // 128
            sidx = cp.tile([128, NTT_], i32, name="sidx")
            nc.sync.dma_start(sidx[:], sent[:, 0:NTT_])
            wb = {}
            for layer in (1, 2):
                wb[layer] = cp.tile([16, 128], bf16, tag=f"wb{layer}",
                                    name=f"swb{layer}")
                nc.sync.dma_start(wb[layer][:], wb_d[layer][:])
            # --- all LSTM weights, preloaded upfront; L1's first so its
            # first steps aren't stuck behind the (bigger) L2 loads ---
            wih1, whh = {}, {}
            for i, cell in enumerate(("1f", "1b")):
                wt = cp.tile([EMB, G4], bf16, tag=f"wih{cell}",
                             name=f"swih{cell}")
                nc.sync.dma_start(wt[:], w_in[f"wih{cell}"][:])
                wih1[i] = wt
            for cell in ("1f", "1b", "2f", "2b"):
                hh = []
                for k in range(2):
                    ht = cp.tile([128, G4], bf16, tag=f"whh{cell}{k}",
                                 name=f"swhh{cell}{k}")
                    nc.sync.dma_start(
                        ht[:], w_in[f"whh{cell}"][128 * k:128 * (k + 1), :])
                    hh.append(ht)
                whh[cell] = hh
            wih2 = {}
            for i, cell in enumerate(("2f", "2b")):
                ch = []
                for k in range(4):
                    wt = cp.tile([128, G4], bf16, tag=f"wih{cell}{k}",
                                 name=f"swih{cell}{k}")
                    nc.sync.dma_start(
                        wt[:], w_in[f"wih{cell}"][128 * k:128 * (k + 1), :])
                    ch.append(wt)
                wih2[i] = ch
            wout_ch = []
            for k in range(4):
                wt = cp.tile([128, TAGS], bf16, tag=f"wout{k}", name=f"swout{k}")
                nc.sync.dma_start(wt[:], wout_d[128 * k:128 * (k + 1), :])
                wout_ch.append(wt)
            bout = cp.tile([1, TAGS], bf16)
            nc.sync.dma_start(bout[:], bout_d[:])
            whh1 = {0: whh["1f"], 1: whh["1b"]}
            whh2 = {0: whh["2f"], 1: whh["2b"]}

            def lstm_layer(tc, layer, pre_mms, whhl, histf, histb, T,
                           post_step=None, pre_step=None):
                """T steps, f+b cells packed.  pre_mms(gpsl, s, cc, t, gc)
                issues the input-projection matmuls for one gate-chunk slice.
                Gate col layout: gate(o,i,f,g)*64 + cc*32 + hc*16 + b.
                h' is written straight into histf/histb (no scratch: slices
                are written once, so the writes carry no WAR waits)."""
                with tc.tile_pool(name=f"l{layer}_work", bufs=SG_BUFS) as lp, \
                        tc.tile_pool(name=f"l{layer}_cp", bufs=3) as cpp, \
                        tc.tile_pool(name=f"l{layer}_psum", bufs=PSUM_BUFS,
                                     space="PSUM") as pp:
                    c_prev = None
                    hist = {0: histf, 1: histb}
                    for s in range(T):
                        if pre_step is not None:
                            pre_step(s)
                        ts_ = {0: s, 1: T - 1 - s}
                        gp = pp.tile([128, 256], f32, tag="gp", name="gp")
                        # bias: gp[p, col] = beta (K=16 selector matmul)
                        nc.tensor.matmul(gp[:], wb[layer][:], bsel[:],
                                         start=True, stop=False,
                                         skip_group_check=True)
                        # input projection
                        for cc in (0, 1):
                            t = ts_[cc]
                            for gc in range(8):
                                gt, hc = gc // 2, gc % 2
                                csl = slice(gt * 64 + cc * 32 + hc * 16,
                                            gt * 64 + cc * 32 + hc * 16 + 16)
                                pre_mms(gp[:, csl], s, cc, t, gc)
                        # hh: f-cell block first (h'_f lands before h'_b)
                        if s > 0:
                            for cc in (0, 1):
                                tp = ts_[cc] + (1 if cc else -1)
                                for gc in range(8):
                                    gt, hc = gc // 2, gc % 2
                                    cbase = gt * 64 + cc * 32 + hc * 16
                                    csl = slice(cbase, cbase + 16)
                                    for kc in range(2):
                                        nc.tensor.matmul(
                                            gp[:, csl],
                                            whhl[cc][kc][:, gc * 128:(gc + 1) * 128],
                                            hist[cc][:, tp * 32 + kc * 16:
                                                      tp * 32 + kc * 16 + 16],
                                            start=False, stop=(kc == 1),
                                            skip_group_check=True)
                        # ---- pointwise ----
                        sgi = lp.tile([128, 192], f32, tag="sgi", name="sgi")
                        nc.scalar.activation(sgi[:], gp[:, 64:256], SIG)
                        sgo = lp.tile([128, 64], f32, tag="sgo", name="sgo")
                        nc.scalar.activation(sgo[:], gp[:, 0:64], SIG)
                        ap_t = lp.tile([128, 64], f32, tag="ap")
                        if s == 0:
                            nc.vector.scalar_tensor_tensor(
                                ap_t[:], sgi[:, 128:192], 0.5, sgi[:, 0:64],
                                SUB, MUL)
                            c_new = cpp.tile([128, 64], f32, tag="c", name="c")
                            nc.vector.tensor_scalar(c_new[:], ap_t[:], 2.0,
                                                    None, MUL)
                        else:
                            b_t = lp.tile([128, 64], f32, tag="bb")
                            nc.vector.tensor_tensor(
                                b_t[:], sgi[:, 64:128], c_prev[:], MUL)
                            nc.vector.scalar_tensor_tensor(
                                ap_t[:], sgi[:, 128:192], 0.5, sgi[:, 0:64],
                                SUB, MUL)
                            c_new = cpp.tile([128, 64], f32, tag="c", name="c")
                            nc.vector.scalar_tensor_tensor(
                                c_new[:], ap_t[:], 2.0, b_t[:], MUL, ADD)
                        c_prev = c_new
                        tcp = lp.tile([128, 64], f32, tag="tc")
                        nc.scalar.activation(tcp[:], c_new[:], SIG, scale=2.0)
                        for cc in (0, 1):
                            t = ts_[cc]
                            nc.vector.scalar_tensor_tensor(
                                hist[cc][:, t * 32:t * 32 + 32],
                                tcp[:, cc * 32:cc * 32 + 32], 0.5,
                                sgo[:, cc * 32:cc * 32 + 32], SUB, MUL)
                        if post_step is not None:
                            post_step(s)

            # ================= hist + embT =================
            with tc.tile_pool(name="hist", bufs=1) as hp:
                h1f = hp.tile([128, 32 * T], bf16, tag="h1f", name="h1f")
                h1b = hp.tile([128, 32 * T], bf16, tag="h1b", name="h1b")
                h2f = hp.tile([128, 32 * T], bf16, tag="h2f", name="h2f")
                h2b = hp.tile([128, 32 * T], bf16, tag="h2b", name="h2b")

                with tc.tile_pool(name="l1_fix", bufs=1) as p_l1:
                    embT = p_l1.tile([128, NTOK], bf16)
                    # gather ends-inward, interleaved with the L1 steps, so
                    # L1 (f from tile 0, b from tile 31) starts immediately
                    # and the middle tiles stream in ahead of consumption
                    with tc.tile_pool(name="embp", bufs=4) as ep, \
                            tc.tile_pool(name="embpp", bufs=2,
                                         space="PSUM") as epp:
                        def gather(g):
                            et = ep.tile([128, EMB], bf16, tag="et")
                            nc.gpsimd.indirect_dma_start(
                                out=et[:], out_offset=None,
                                in_=emb_d[:],
                                in_offset=bass.IndirectOffsetOnAxis(
                                    ap=sidx[:, g:g + 1], axis=0),
                            )
                            etp = epp.tile([128, EMB], bf16, tag="etp")
                            nc.tensor.transpose(etp[:], et[:], ident[:])
                            nc.vector.tensor_copy(
                                embT[:, 128 * g:128 * (g + 1)], etp[:])

                        for g in (0, NTT - 1, 1, NTT - 2):
                            gather(g)

                        def pre_step1(s):
                            if s % 8 == 0 and s // 8 + 2 <= NTT // 2 - 1:
                                k = s // 8
                                gather(k + 2)
                                gather(NTT - 3 - k)

                        def pre1(gpsl, s, cc, t, gc):
                            nc.tensor.matmul(
                                gpsl, wih1[cc][:, gc * 128:(gc + 1) * 128],
                                embT[:, t * 16:t * 16 + 16],
                                start=False, stop=False,
                                skip_group_check=True)

                        # inside the gather pools: no drain barrier, so L1
                        # starts as soon as the edge tiles land
                        lstm_layer(tc, 1, pre1, whh1, h1f, h1b, T,
                                   pre_step=pre_step1)
                # p_l1 closed: embT freed

                # ================= L2 (+ interleaved OUT) =================
                hist1 = {0: h1f, 1: h1b}
                hist2 = {0: h2f, 1: h2b}

                def pre2(gpsl, s, cc, t, gc):
                    for kc in range(4):
                        src = hist1[kc // 2]
                        nc.tensor.matmul(
                            gpsl,
                            wih2[cc][kc][:, gc * 128:(gc + 1) * 128],
                            src[:, t * 32 + (kc % 2) * 16:
                                t * 32 + (kc % 2) * 16 + 16],
                            start=False, stop=False, skip_group_check=True)

                # OUT tile g is ready after L2 step max(8g+7, 255-8g)
                ready = {}
                for g in range(NTT):
                    ready.setdefault(max(8 * g + 7, T - 1 - 8 * g), []).append(g)

                with tc.tile_pool(name="outw", bufs=3) as ow, \
                        tc.tile_pool(name="outp", bufs=2, space="PSUM") as op:

                    def out_tile(g):
                        # transposed: ps[tag, token] so per-step results land
                        # in free-dim column slices (no partition-base limits)
                        ps = op.tile([TAGS, 128], f32, tag="ops")
                        nc.tensor.matmul(ps[:], bout[:1, :], onescol[:1, :],
                                         start=True, stop=False,
                                         skip_group_check=True)
                        for tau in range(8):
                            t = g * 8 + tau
                            for k in range(4):
                                cc, hc = k // 2, k % 2
                                nc.tensor.matmul(
                                    ps[:, tau * 16:tau * 16 + 16],
                                    wout_ch[k][:],
                                    hist2[cc][:, t * 32 + hc * 16:
                                              t * 32 + hc * 16 + 16],
                                    start=False, stop=(k == 3),
                                    skip_group_check=True)
                        sb = ow.tile([TAGS, 128], f32, tag="osb")
                        nc.vector.tensor_copy(sb[:], ps[:])
                        nc.sync.dma_start(out_d[:, 128 * g:128 * (g + 1)],
                                          sb[:])

                    def post_step(s):
                        for g in ready.get(s, ()):
                            out_tile(g)

                    lstm_layer(tc, 2, pre2, whh2, h2f, h2b, T,
                               post_step=post_step)

    return nc


def _prep_cell_weights(wih, whh, bih, bhh, h_in_scale):
    """Gate perm i,f,g,o -> o,i,f,g; scale g rows x2 (tanh(x)=2*sig(2x)-1);
    whh rows x2 and wih rows x h_in_scale compensate the h'=h/2 storage.
    Returns (wihT, whhT, brow) f32 with transposed [Din, 4H] layout."""
    H = HID
    idx = np.concatenate([np.arange(3 * H, 4 * H),      # o
                          np.arange(0, H),              # i
                          np.arange(H, 2 * H),          # f
                          np.arange(2 * H, 3 * H)])     # g
    gscale = np.ones((4 * H, 1), np.float32)
    gscale[3 * H:4 * H] = 2.0
    wih_p = wih[idx] * gscale * h_in_scale
    whh_p = whh[idx] * gscale * 2.0
    b_p = (bih + bhh)[idx] * gscale[:, 0]
    return (np.ascontiguousarray(wih_p.T, np.float32),
            np.ascontiguousarray(whh_p.T, np.float32),
            np.ascontiguousarray(b_p, np.float32))


class Runner:
    """Build the SPMD program once; execute repeatedly on device-resident
    inputs (for clean timing, no donation so buffers are reusable)."""

    def __init__(self, nc, n_cores=NCORES):
        import jax
        import numpy as _np
        from jax.sharding import Mesh, PartitionSpec
        from jax.experimental.shard_map import shard_map
        import concourse.mybir as mybir
        from concourse import bass2jax as b2j

        b2j.install_neuronx_cc_hook()
        self.jax = jax
        self.nc = nc
        self.n_cores = n_cores
        partition_name = (nc.partition_id_tensor.name
                          if nc.partition_id_tensor else None)
        in_names, out_names, out_avals, zero_outs = [], [], [], []
        for alloc in nc.m.functions[0].allocations:
            if not isinstance(alloc, mybir.MemoryLocationSet):
                continue
            name = alloc.memorylocations[0].name
            if alloc.kind == "ExternalInput":
                if name != partition_name:
                    in_names.append(name)
            elif alloc.kind == "ExternalOutput":
                out_names.append(name)
                shape = tuple(alloc.tensor_shape)
                dtype = mybir.dt.np(alloc.dtype)
                out_avals.append(jax.core.ShapedArray(shape, dtype))
                zero_outs.append(_np.zeros(shape, dtype))
        self.n_params = len(in_names)
        self.in_names = list(in_names)
        self.out_names = list(out_names)
        self.out_avals = out_avals
        self.zero_outs = zero_outs
        all_in = in_names + out_names
        if partition_name is not None:
            all_in.append(partition_name)

        def _body(*args):
            operands = list(args)
            if partition_name is not None:
                operands.append(b2j.partition_id_tensor())
            outs = b2j._bass_exec_p.bind(
                *operands,
                out_avals=tuple(out_avals),
                in_names=tuple(all_in),
                out_names=tuple(out_names),
                lowering_input_output_aliases=(),
                sim_require_finite=True,
                sim_require_nnan=True,
                nc=nc,
            )
            return tuple(outs)

        devices = jax.devices()[:n_cores]
        self.mesh = Mesh(_np.asarray(devices), ("core",))
        in_specs = (PartitionSpec("core"),) * (self.n_params + len(out_names))
        out_specs = (PartitionSpec("core"),) * len(out_names)
        self.sharded = jax.jit(shard_map(_body, mesh=self.mesh,
                                         in_specs=in_specs,
                                         out_specs=out_specs, check_rep=False),
                               keep_unused=True)
        self.dev_args = None

    def put(self, in_maps):
        """Upload per-core input maps as device-sharded global arrays."""
        import numpy as _np
        from jax.sharding import NamedSharding, PartitionSpec
        jax = self.jax
        sh = NamedSharding(self.mesh, PartitionSpec("core"))
        args = []
        for name in self.in_names:
            g = _np.concatenate([_np.asarray(m[name]) for m in in_maps], axis=0)
            args.append(jax.device_put(g, sh))
        for z in self.zero_outs:
            g = _np.zeros((self.n_cores * z.shape[0],) + z.shape[1:], z.dtype)
            args.append(jax.device_put(g, sh))
        self.dev_args = args

    def run(self):
        outs = self.sharded(*self.dev_args)
        self.jax.block_until_ready(outs)
        return outs

    def results(self, outs):
        import numpy as _np
        res = []
        for c in range(self.n_cores):
            res.append({name: _np.asarray(outs[i]).reshape(
                (self.n_cores,) + self.out_avals[i].shape)[c]
                for i, name in enumerate(self.out_names)})
        return res

    def time_exec(self, iters=10):
        import time as _time
        self.run()  # warm
        best = float("inf")
        for _ in range(iters):
            t0 = _time.perf_counter()
            self.run()
            best = min(best, _time.perf_counter() - t0)
        return best


_RUNNERS = {}


def get_runner(T=T_FULL):
    if T not in _RUNNERS:
        _RUNNERS[T] = Runner(build_program(T))
    return _RUNNERS[T]


def make_in_maps(sentence, emb,
                 wih1f, whh1f, bih1f, bhh1f,
                 wih1b, whh1b, bih1b, bhh1b,
                 wih2f, whh2f, bih2f, bhh2f,
                 wih2b, whh2b, bih2b, bhh2b,
                 w_out, b_out, T=T_FULL):
    NTOK = BL * T
    NTT = NTOK // 128
    bf = ml_dtypes.bfloat16

    # selector S[k, col]: k = gt*4 + cc*2 + hc ; col = gt*64 + cc*32 + hc*16 + b
    S = np.zeros((16, 256), np.float32)
    for gt in range(4):
        for cci in range(2):
            for hc in range(2):
                k = gt * 4 + cci * 2 + hc
                base = gt * 64 + cci * 32 + hc * 16
                S[k, base:base + 16] = 1.0

    common = {
        "emb": np.asarray(emb, np.float32).astype(bf),
        "ident128b": np.eye(128).astype(bf),
        "onescol": np.ones((1, 128), np.float32).astype(bf),
        "bsel": S.astype(bf),
        "woutT": np.ascontiguousarray(
            np.asarray(w_out, np.float32).T * 2.0).astype(bf),
        "bout": np.asarray(b_out, np.float32).reshape(1, TAGS).astype(bf),
    }
    brows = {}
    for cell, (wi, wh, bi, bh, hin) in {
        "1f": (wih1f, whh1f, bih1f, bhh1f, 1.0),
        "1b": (wih1b, whh1b, bih1b, bhh1b, 1.0),
        "2f": (wih2f, whh2f, bih2f, bhh2f, 2.0),
        "2b": (wih2b, whh2b, bih2b, bhh2b, 2.0),
    }.items():
        wihT, whhT, brow = _prep_cell_weights(
            np.asarray(wi, np.float32), np.asarray(wh, np.float32),
            np.asarray(bi, np.float32), np.asarray(bh, np.float32), hin)
        common[f"wih{cell}"] = wihT.astype(bf)
        common[f"whh{cell}"] = whhT.astype(bf)
        brows[cell] = brow
    # WbT[k, p] = beta_cell[gt*256 + hc*128 + p],  k = gt*4 + cc*2 + hc
    for layer, (cf, cb) in ((1, ("1f", "1b")), (2, ("2f", "2b"))):
        Wb = np.zeros((16, 128), np.float32)
        for gt in range(4):
            for cci, cell in enumerate((cf, cb)):
                for hc in range(2):
                    k = gt * 4 + cci * 2 + hc
                    Wb[k, :] = brows[cell][gt * 256 + hc * 128:
                                           gt * 256 + hc * 128 + 128]
        common[f"wb{layer}"] = Wb.astype(bf)

    sentence = np.asarray(sentence)
    in_maps = []
    for c in range(NCORES):
        sl = sentence[c * BL:(c + 1) * BL, :T]
        flat = np.ascontiguousarray(sl.T).reshape(NTOK)
        sent_in = np.ascontiguousarray(
            flat.reshape(NTT, 128).T.astype(np.int32))
        m = dict(common)
        m["sent"] = sent_in
        in_maps.append(m)
    return in_maps


def kernel(sentence, emb,
           wih1f, whh1f, bih1f, bhh1f,
           wih1b, whh1b, bih1b, bhh1b,
           wih2f, whh2f, bih2f, bhh2f,
           wih2b, whh2b, bih2b, bhh2b,
           w_out, b_out, _T=T_FULL):
    T = _T
    rn = get_runner(T)
    in_maps = make_in_maps(sentence, emb,
                           wih1f, whh1f, bih1f, bhh1f,
                           wih1b, whh1b, bih1b, bhh1b,
                           wih2f, whh2f, bih2f, bhh2f,
                           wih2b, whh2b, bih2b, bhh2b,
                           w_out, b_out, T=T)
    rn.put(in_maps)
    outs = rn.run()
    res = rn.results(outs)
    full = np.concatenate(
        [res[c]["out"].reshape(TAGS, T, BL).transpose(2, 1, 0)
         for c in range(NCORES)], axis=0)
    return full
